# revision 10
# baseline (speedup 1.0000x reference)
"""SoftRas-style soft rasterizer on 8 Trainium2 NeuronCores — v2.

Structure (vs the v1 baseline, ~3.7x fewer face-pixel ops):
- Pixel tiles are 16x32 px (128 tiles); faces from *different* tiles are
  packed into shared 128-row chunks by expressing each face's affine
  coefficients in its tile's local pixel frame (basis [1, lx, ly] is
  bf16-exact).  This removes the pad-to-128 quantization of v1.
- Two populations: most (face,tile) pairs only matter for the alpha
  channel (coverage product) and run a short chain: signed distance ->
  softplus.  Only faces within a per-tile adaptive score threshold of the
  pixel-depth softmax peak run the full barycentric/depth/RGB chain.
- Per-tile accumulators (rgb x3, dsum, ln-alpha) live in one [80, 512]
  PSUM bank, fed by block-diagonal stationary matmuls.
- softplus(a) = Ln(1 + Exp(a)) keeps every Act LUT op in one table set
  (natural_log_exp); sigmoid is eliminated via prob*E = exp(a + l + zarg).
- Sign of the distance: S = max_e sLD_e via a transposed-AP tensor_reduce,
  turned into +-1.0f with one tensor_scalar bit-trick, applied by a Pool
  multiply.
"""
import sys
sys.path.insert(0, '/opt/trn_rl_repo')
import numpy as np
import ml_dtypes
from contextlib import ExitStack

import concourse.bass as bass
import concourse.bacc as bacc
import concourse.tile as tile
import concourse.mybir as mybir
from concourse.bass_utils import run_bass_kernel_spmd

TRACE = False
LAST_RESULT = None

F_TOT = 512
H = W = 256
NCORES = 8
TR, TC = 16, 32                  # tile rows/cols
TP = TR * TC                     # 512 px per tile
NTY, NTX = H // TR, W // TC      # 16 x 8 tiles
NT = NTY * NTX                   # 128 tiles
TPC = NT // NCORES               # 16 tiles per core
SIGMA = 1e-2
GAMMA = 1e-3
EPS = 1e-3
NEAR, FAR = 1.0, 100.0
GAP_A = 0.095                    # alpha keep radius (p ~ 7.5e-5)
MARGIN = 14.0                    # rgb score margin (e^-14 ~ 8e-7)

FP = mybir.dt.float32
F32R = mybir.dt.float32r
BF = mybir.dt.bfloat16
U32 = mybir.dt.uint32
AL = mybir.AluOpType
AF = mybir.ActivationFunctionType

BF16 = ml_dtypes.bfloat16


def _split2(a):
    h = np.asarray(a, np.float64).astype(BF16).astype(np.float64)
    return h, (a - h)


def _host_prep(face_vertices):
    fv = np.asarray(face_vertices, np.float64)[0]          # [F,3,3]
    F = fv.shape[0]
    x = fv[:, :, 0]; y = fv[:, :, 1]; z = fv[:, :, 2]
    x0, x1, x2 = x[:, 0], x[:, 1], x[:, 2]
    y0, y1, y2 = y[:, 0], y[:, 1], y[:, 2]

    den = (y1 - y2) * (x0 - x2) + (x2 - x1) * (y0 - y2)
    den = np.where(np.abs(den) < 1e-10, 1e-10, den)
    # barycentric coefficient rows [c0, cx, cy] per k
    Wc = np.zeros((F, 3, 3))
    Wc[:, 0] = np.stack([(-(y1 - y2) * x2 - (x2 - x1) * y2) / den,
                         (y1 - y2) / den, (x2 - x1) / den], -1)
    Wc[:, 1] = np.stack([(-(y2 - y0) * x2 - (x0 - x2) * y2) / den,
                         (y2 - y0) / den, (x0 - x2) / den], -1)
    Wc[:, 2] = -Wc[:, 0] - Wc[:, 1]
    Wc[:, 2, 0] += 1.0                                      # w2 = 1-w0-w1

    anchors = [(x0, y0), (x1, y1), (x2, y2)]
    pairs = [(0, 1), (1, 2), (2, 0)]
    # per edge: U' = ((p-a).d)/L - L/2  (along-line, centered)
    #           sLD = cross(p-a, d)/L oriented positive OUTSIDE
    Uc = np.zeros((F, 3, 3)); Lc = np.zeros((F, 3, 3)); HL = np.zeros((F, 3))
    cx = (x0 + x1 + x2) / 3.0; cy = (y0 + y1 + y2) / 3.0
    for e, (ia, ib) in enumerate(pairs):
        ax, ay = anchors[ia]; bx, by = anchors[ib]
        dx, dy = bx - ax, by - ay
        L = np.sqrt(np.maximum(dx * dx + dy * dy, 1e-12))
        iL = 1.0 / L
        Uc[:, e, 0] = (-ax * dx - ay * dy) * iL - L / 2.0
        Uc[:, e, 1] = dx * iL
        Uc[:, e, 2] = dy * iL
        sc0 = (ay * dx - ax * dy) * iL
        scx = dy * iL
        scy = -dx * iL
        v = sc0 + scx * cx + scy * cy                      # value at centroid
        orient = np.where(np.abs(v) < 1e-12, 1.0, -np.sign(v))
        Lc[:, e, 0] = sc0 * orient
        Lc[:, e, 1] = scx * orient
        Lc[:, e, 2] = scy * orient
        HL[:, e] = L / 2.0
    iz = 1.0 / z
    assert z.min() > NEAR + 0.05 and z.max() < FAR - 0.05
    # inradius bound: max inside distance -> exp(a) stays finite
    assert HL.max() < 0.6, "exp overflow guard"
    znUB = (FAR - z.min(1)) / (FAR - NEAR)
    znLB = (FAR - z.max(1)) / (FAR - NEAR)
    return dict(Wc=Wc, Uc=Uc, Lc=Lc, HL=HL, iz=iz,
                ymin=y.min(1), ymax=y.max(1), xmin=x.min(1), xmax=x.max(1),
                znUB=znUB, znLB=znLB,
                x=x, y=y, z=z, den=den)


def _tile_thresholds(prep, mhat):
    """Per-tile rgb keep-threshold: max over tile pixels of the min face
    score, + margin.  Scores evaluated exactly for the near-depth subset."""
    D = prep['znLB'].max()
    sub = np.nonzero(prep['znUB'] >= mhat - 0.1)[0]
    assert len(sub) > 0
    pix = ((np.arange(H, dtype=np.float64) + 0.5) / H) * 2.0 - 1.0
    px = pix[None, None, :]; py = pix[None, :, None]
    x = prep['x'][sub]; y = prep['y'][sub]; z = prep['z'][sub]

    best = np.full((H, W), 1e30)
    for s in range(0, len(sub), 64):
        sl = slice(s, min(s + 64, len(sub)))
        X0, X1, X2 = (x[sl, k][:, None, None] for k in range(3))
        Y0, Y1, Y2 = (y[sl, k][:, None, None] for k in range(3))
        den = (y[sl, 1] - y[sl, 2]) * (x[sl, 0] - x[sl, 2]) + \
              (x[sl, 2] - x[sl, 1]) * (y[sl, 0] - y[sl, 2])
        den = np.where(np.abs(den) < 1e-10, 1e-10, den)[:, None, None]
        w0 = ((Y1 - Y2) * (px - X2) + (X2 - X1) * (py - Y2)) / den
        w1 = ((Y2 - Y0) * (px - X2) + (X0 - X2) * (py - Y2)) / den
        w2 = 1.0 - w0 - w1
        inside = (w0 >= 0) & (w1 >= 0) & (w2 >= 0)

        def ed2(ax, ay, bx, by):
            dx, dy = bx - ax, by - ay
            l2 = np.maximum(dx * dx + dy * dy, 1e-12)
            t = np.clip(((px - ax) * dx + (py - ay) * dy) / l2, 0.0, 1.0)
            return (px - (ax + t * dx)) ** 2 + (py - (ay + t * dy)) ** 2

        d2 = np.minimum(np.minimum(ed2(X0, Y0, X1, Y1), ed2(X1, Y1, X2, Y2)),
                        ed2(X2, Y2, X0, Y0))
        dist = np.sqrt(d2 + 1e-12)
        wc0, wc1, wc2 = np.clip(w0, 0, 1), np.clip(w1, 0, 1), np.clip(w2, 0, 1)
        ssum = np.maximum(wc0 + wc1 + wc2, 1e-12)
        izl = 1.0 / z[sl]
        P = (wc0 * izl[:, 0, None, None] + wc1 * izl[:, 1, None, None]
             + wc2 * izl[:, 2, None, None]) / ssum
        zp = 1.0 / np.maximum(P, 1e-12)
        zn = np.clip((FAR - zp) / (FAR - NEAR), 0, 1)
        a = np.where(inside, dist, -dist) / SIGMA
        score = np.logaddexp(0.0, -a) + (mhat - zn) / GAMMA
        best = np.minimum(best, score.min(axis=0))
    Sstar = best.max()
    assert Sstar < 80.0, f"fp32 dsum underflow risk: S*={Sstar}"
    tb = best.reshape(NTY, TR, NTX, TC).max(axis=(1, 3))    # [NTY, NTX]
    return tb + MARGIN, D


def _populate(prep, Tt, D, mhat):
    """FULL / ALPHA face lists per tile + core assignment + chunk pattern."""
    pix = ((np.arange(H, dtype=np.float64) + 0.5) / H) * 2.0 - 1.0
    y0v = pix[np.arange(NTY) * TR]; y1v = pix[np.arange(NTY) * TR + TR - 1]
    x0v = pix[np.arange(NTX) * TC]; x1v = pix[np.arange(NTX) * TC + TC - 1]
    ygap = np.maximum(0.0, np.maximum(prep['ymin'][None] - y1v[:, None],
                                      y0v[:, None] - prep['ymax'][None]))
    xgap = np.maximum(0.0, np.maximum(prep['xmin'][None] - x1v[:, None],
                                      x0v[:, None] - prep['xmax'][None]))
    gap = np.sqrt(ygap[:, None, :] ** 2 + xgap[None, :, :] ** 2)  # [NTY,NTX,F]
    sbound = gap / SIGMA + (D - prep['znUB'])[None, None] / GAMMA
    keep_full = sbound < Tt[:, :, None]
    keep_alpha = (gap < GAP_A) & ~keep_full
    nf = keep_full.sum(2).ravel(); na = keep_alpha.sum(2).ravel()

    # assign 16 tiles/core: balance full pairs (hard-ish) then alpha
    order = np.argsort(-(nf * 4 + na), kind='stable')
    loads_f = np.zeros(NCORES, np.int64)
    loads_a = np.zeros(NCORES, np.int64)
    counts = np.zeros(NCORES, np.int64)
    assign = [[] for _ in range(NCORES)]
    for t in order:
        cands = [c for c in range(NCORES) if counts[c] < TPC]
        c = min(cands, key=lambda c: (loads_f[c] * 4 + loads_a[c]))
        assign[c].append(int(t))
        loads_f[c] += nf[t]; loads_a[c] += na[t]; counts[c] += 1
    NF = int(np.ceil(loads_f.max() / 128))
    NA = int(np.ceil(loads_a.max() / 128))
    kf = keep_full.reshape(NT, F_TOT); ka = keep_alpha.reshape(NT, F_TOT)
    return kf, ka, assign, NF, NA


def _pack_slots(assign_c, kf, ka, NF, NA):
    """Per-core slot lists: [(tile_local, face)] padded with (-1,-1)."""
    fulls, alphas = [], []
    for tl, t in enumerate(assign_c):
        for f in np.nonzero(kf[t])[0]:
            fulls.append((tl, int(f)))
        for f in np.nonzero(ka[t])[0]:
            alphas.append((tl, int(f)))
    assert len(fulls) <= NF * 128 and len(alphas) <= NA * 128
    fulls += [(-1, -1)] * (NF * 128 - len(fulls))
    alphas += [(-1, -1)] * (NA * 128 - len(alphas))
    return np.array(fulls + alphas, np.int64).reshape(NF + NA, 128, 2)


def _build_inputs(prep, textures, slots, assign_c, NF, NA):
    """All dram input arrays for one core."""
    NC = NF + NA
    pix = ((np.arange(H, dtype=np.float64) + 0.5) / H) * 2.0 - 1.0
    tl = slots[:, :, 0]; fi = slots[:, :, 1]            # [NC,128]
    dummy = fi < 0
    tlz = np.where(dummy, 0, tl); fiz = np.where(dummy, 0, fi)
    tg = np.array(assign_c)[tlz]                        # global tile id
    ty, tx = tg // NTX, tg % NTX
    ox = pix[tx * TC]; oy = pix[ty * TR]                # [NC,128]

    def localize(C):                                    # C: [F,3,3] k/e-major
        c = C[fiz]                                      # [NC,128,3,3]
        c0 = c[..., 0] + c[..., 1] * ox[..., None] + c[..., 2] * oy[..., None]
        return np.stack([c0, c[..., 1], c[..., 2]], -1)  # [NC,128,3,3]

    ld = localize(prep['Lc'])
    u = localize(prep['Uc'])
    ld[dummy] = 0.0; u[dummy] = 0.0
    ld[dummy, :, 0] = 10.0                              # far outside
    h = prep['HL'][fiz]; h[dummy] = 0.5                 # [NC,128,3]

    ldh, ldl = _split2(ld); uh, ul = _split2(u)
    st_ld = np.concatenate([ldh, ldl], axis=-1)         # [NC,128,3,6]
    st_u = np.concatenate([uh, ul], axis=-1)
    st_ld = st_ld.transpose(0, 3, 2, 1)                 # [NC,6,3,128]
    st_u = st_u.transpose(0, 3, 2, 1)

    w = localize(prep['Wc'])[:NF] if NF else np.zeros((0, 128, 3, 3))
    dw = dummy[:NF]
    w[dw] = 0.0
    w[dw, 0, 0] = -1.0; w[dw, 1, 0] = -1.0; w[dw, 2, 0] = 3.0
    w1h, r = _split2(w); w2h, w3h = _split2(r)
    st_w = np.concatenate([w1h, w2h, w3h], axis=-1)     # [NF,128,3,9]
    st_w = st_w.transpose(0, 3, 2, 1)                   # [NF,9,3,128]

    izf = prep['iz'][fiz[:NF]] if NF else np.zeros((0, 128, 3))
    izf[dummy[:NF]] = 0.011

    # accumulator stationaries: acc row layout per local tile: 5*tl + {0..4}
    ln_st = np.zeros((NC, 128, 5 * TPC))
    ln_st[np.arange(NC)[:, None], np.arange(128)[None], 5 * tlz + 4] = \
        (~dummy).astype(np.float64)

    g_st = np.zeros((NF, 3, 128, 5 * TPC))
    if NF:
        tex = np.asarray(textures, np.float64)[0][fiz[:NF]]   # [NF,128,3,3]
        tex[dummy[:NF]] = 0.0
        ar = np.arange(128)[None]
        for k in range(3):
            for c in range(3):
                g_st[np.arange(NF)[:, None], k, ar, 5 * tlz[:NF] + c] = \
                    tex[:, :, k, c]
            g_st[np.arange(NF)[:, None], k, ar, 5 * tlz[:NF] + 3] = \
                (~dummy[:NF]).astype(np.float64)

    # basis (shared, exact in bf16)
    lx = (np.arange(TP) % TC) / 128.0
    ly = (np.arange(TP) // TC) / 128.0
    b3 = np.stack([np.ones(TP), lx, ly])
    bas2 = np.concatenate([b3, b3])                     # [6,TP]
    bas3 = np.concatenate([b3, b3, b3])                 # [9,TP]

    hs = np.concatenate([h, -h], axis=-1)               # [NC,128,6]
    return {
        "st_ld": st_ld.transpose(1, 0, 2, 3).reshape(6, NC * 3 * 128)
                 .astype(BF16),
        "st_u": st_u.transpose(1, 0, 2, 3).reshape(6, NC * 3 * 128)
                .astype(BF16),
        "st_w": st_w.transpose(1, 0, 2, 3).reshape(9, max(NF, 1) * 3 * 128)
                .astype(BF16) if NF else np.zeros((9, 384), BF16),
        "scal": hs.transpose(1, 0, 2).reshape(128, NC * 6).astype(np.float32),
        "izs": izf.transpose(1, 0, 2).reshape(128, max(NF, 1) * 3)
               .astype(np.float32) if NF else np.zeros((128, 3), np.float32),
        "ln_st": ln_st.transpose(1, 0, 2).reshape(128, NC * 5 * TPC)
                 .astype(BF16),
        "g_st": g_st.transpose(2, 0, 1, 3).reshape(128, max(NF, 1) * 3 * 5 * TPC)
                .astype(BF16) if NF else np.zeros((128, 240), BF16),
        "bas2": bas2.astype(BF16),
        "bas3": bas3.astype(BF16),
    }


def _steer_act_tables():
    import functools
    import concourse.hw_specs as hw_specs
    real = hw_specs.get_activation_tables
    if getattr(real, '_steered', False):
        return
    base = real.__wrapped__ if hasattr(real, '__wrapped__') else real

    @functools.cache
    def steered(arch):
        t = dict(base(arch))
        for name in ('exp_and_others', 'natural_log', 'exp_and_friends'):
            if name in t and 'natural_log_exp_and_others' in t:
                t[name] = set()
        return t

    steered._steered = True
    hw_specs.get_activation_tables = steered
    bacc.get_activation_tables = steered


def _build_program(NF, NA, mhat):
    _steer_act_tables()
    NC = NF + NA
    ACC = 5 * TPC                                        # 80
    nc = bacc.Bacc("TRN2", target_bir_lowering=False, debug=False,
                   num_devices=NCORES)
    d_stld = nc.dram_tensor("st_ld", [6, NC * 3 * 128], BF,
                            kind="ExternalInput")
    d_stu = nc.dram_tensor("st_u", [6, NC * 3 * 128], BF,
                           kind="ExternalInput")
    d_stw = nc.dram_tensor("st_w", [9, max(NF, 1) * 3 * 128], BF,
                           kind="ExternalInput")
    d_scal = nc.dram_tensor("scal", [128, NC * 6], FP, kind="ExternalInput")
    d_izs = nc.dram_tensor("izs", [128, max(NF, 1) * 3], FP,
                           kind="ExternalInput")
    d_lnst = nc.dram_tensor("ln_st", [128, NC * ACC], BF,
                            kind="ExternalInput")
    d_gst = nc.dram_tensor("g_st", [128, max(NF, 1) * 3 * ACC], BF,
                           kind="ExternalInput")
    d_bas2 = nc.dram_tensor("bas2", [6, TP], BF, kind="ExternalInput")
    d_bas3 = nc.dram_tensor("bas3", [9, TP], BF, kind="ExternalInput")
    d_out = nc.dram_tensor("out", [ACC, TP], FP, kind="ExternalOutput")

    bias_pe = float(FAR / (FAR - NEAR) / GAMMA - 1000.0 * mhat)
    zsc = float(-1.0 / ((FAR - NEAR) * GAMMA))

    with ExitStack() as ctx:
        tc = ctx.enter_context(tile.TileContext(nc))
        const = ctx.enter_context(tc.tile_pool(name="const", bufs=1))
        tr1 = ctx.enter_context(tc.tile_pool(name="tr1", bufs=3))
        tr2 = ctx.enter_context(tc.tile_pool(name="tr2", bufs=3))
        keep = ctx.enter_context(tc.tile_pool(name="keep", bufs=NC))
        keepf = ctx.enter_context(tc.tile_pool(name="keepf", bufs=max(NF, 1)))
        qp = ctx.enter_context(tc.tile_pool(name="qp", bufs=2, space="PSUM"))
        accp = ctx.enter_context(tc.tile_pool(name="accp", bufs=1, space="PSUM"))

        st_ld = const.tile([6, NC, 3, 128], BF)
        nc.sync.dma_start(out=st_ld.rearrange("k c e f -> k (c e f)"),
                          in_=d_stld[:, :])
        st_u = const.tile([6, NC, 3, 128], BF)
        nc.sync.dma_start(out=st_u.rearrange("k c e f -> k (c e f)"),
                          in_=d_stu[:, :])
        st_w = const.tile([9, max(NF, 1), 3, 128], BF)
        nc.sync.dma_start(out=st_w.rearrange("k c e f -> k (c e f)"),
                          in_=d_stw[:, :])
        scal = const.tile([128, NC, 6], FP)
        nc.sync.dma_start(out=scal.rearrange("f c s -> f (c s)"),
                          in_=d_scal[:, :])
        izs = const.tile([128, max(NF, 1), 3], FP)
        nc.sync.dma_start(out=izs.rearrange("f c s -> f (c s)"),
                          in_=d_izs[:, :])
        ln_st = const.tile([128, NC, ACC], BF)
        nc.sync.dma_start(out=ln_st.rearrange("f c a -> f (c a)"),
                          in_=d_lnst[:, :])
        g_st = const.tile([128, max(NF, 1), 3, ACC], BF)
        nc.sync.dma_start(out=g_st.rearrange("f c k a -> f (c k a)"),
                          in_=d_gst[:, :])
        bas2 = const.tile([6, TP], BF)
        nc.sync.dma_start(out=bas2, in_=d_bas2[:, :])
        bas3 = const.tile([9, TP], BF)
        nc.sync.dma_start(out=bas3, in_=d_bas3[:, :])

        b_eps = const.tile([128, 1], FP)
        nc.vector.memset(b_eps, 1e-12)
        b_one = const.tile([128, 1], FP)
        nc.vector.memset(b_one, 1.0)
        b_pe = const.tile([128, 1], FP)
        nc.vector.memset(b_pe, bias_pe)

        acc = accp.tile([ACC, TP], FP, tag="acc")
        ds_all, wc_all, rs_all, zp_all, l_full = {}, {}, {}, {}, {}
        m2_all, m_all = {}, {}

        def emit_sqrt(c):
            d = tr2.tile([128, TP], FP, tag="d")
            nc.scalar.activation(d, m2_all[c], AF.Sqrt, bias=b_eps)
            ds = keep.tile([128, TP], FP, tag="ds", name=f"ds{c}")
            nc.gpsimd.tensor_tensor(out=ds, in0=d, in1=m_all[c], op=AL.mult)
            ds_all[c] = ds

        # ---------------- P1: distance chains (sqrt table) ----------------
        for c in range(NC):
            if c > 0:
                emit_sqrt(c - 1)
            full = c < NF
            ld = qp.tile([128, 3, TP], FP, tag="q", name=f"ld{c}")
            for e in range(3):
                nc.tensor.matmul(ld[:, e, :], st_ld[:, c, e, :], bas2,
                                 start=True, stop=True)
            S = tr1.tile([128, TP], FP, tag="S")
            nc.vector.tensor_reduce(out=S, in_=ld[:, :, :].transpose([0, 2, 1]),
                                    axis=mybir.AxisListType.X, op=AL.max)
            m = keep.tile([128, TP], FP, tag="m", name=f"m_{c}")
            nc.vector.tensor_scalar(out=m.bitcast(U32), in0=S.bitcast(U32),
                                    scalar1=0x80000000, scalar2=0xBF800000,
                                    op0=AL.bitwise_and, op1=AL.bitwise_xor)
            sq = tr1.tile([128, 3, TP], BF, tag="sq")
            nc.scalar.activation(sq.rearrange("f e n -> f (e n)"),
                                 ld.rearrange("f e n -> f (e n)"), AF.Square)
            u = qp.tile([128, 3, TP], FP, tag="q", name=f"u{c}")
            for e in range(3):
                nc.tensor.matmul(u[:, e, :], st_u[:, c, e, :], bas2,
                                 start=True, stop=True)
            aU = tr1.tile([128, 3, TP], BF, tag="aU")
            nc.scalar.activation(aU.rearrange("f e n -> f (e n)"),
                                 u.rearrange("f e n -> f (e n)"), AF.Abs)
            rm = tr1.tile([128, 3, TP], BF, tag="rm")
            for e in range(3):
                nc.gpsimd.tensor_scalar(out=rm[:, e, :], in0=aU[:, e, :],
                                        scalar1=scal[:, c, e:e + 1],
                                        scalar2=scal[:, c, 3 + e:4 + e],
                                        op0=AL.max, op1=AL.add)
            rsq = tr1.tile([128, 3, TP], BF, tag="rsq")
            nc.vector.tensor_tensor(out=rsq.rearrange("f e n -> f (e n)"),
                                    in0=rm.rearrange("f e n -> f (e n)"),
                                    in1=rm.rearrange("f e n -> f (e n)"),
                                    op=AL.mult)
            a3 = tr1.tile([128, 3, TP], BF, tag="a3")
            nc.vector.tensor_tensor(out=a3.rearrange("f e n -> f (e n)"),
                                    in0=rsq.rearrange("f e n -> f (e n)"),
                                    in1=sq.rearrange("f e n -> f (e n)"),
                                    op=AL.add)
            m1 = tr2.tile([128, TP], BF, tag="m1")
            nc.vector.tensor_tensor(out=m1, in0=a3[:, 0, :], in1=a3[:, 1, :],
                                    op=AL.min)
            m2 = keep.tile([128, TP], BF, tag="m2", name=f"m2_{c}")
            nc.vector.tensor_tensor(out=m2, in0=m1, in1=a3[:, 2, :],
                                    op=AL.min)
            m2_all[c], m_all[c] = m2, m

            if full:
                wq = qp.tile([128, 3, TP], FP, tag="q", name=f"w{c}")
                for k in range(3):
                    nc.tensor.matmul(wq[:, k, :], st_w[:, c, k, :], bas3,
                                     start=True, stop=True)
                wc = keepf.tile([128, 3, TP], FP, tag="wc", name=f"wc{c}")
                for k in range(3):
                    nc.vector.tensor_scalar(out=wc[:, k, :], in0=wq[:, k, :],
                                            scalar1=0.0, scalar2=1.0,
                                            op0=AL.max, op1=AL.min)
                s01 = tr2.tile([128, TP], FP, tag="s01")
                nc.gpsimd.tensor_tensor(out=s01, in0=wc[:, 0, :],
                                        in1=wc[:, 1, :], op=AL.add)
                ssum = tr2.tile([128, TP], FP, tag="ssum")
                nc.gpsimd.tensor_tensor(out=ssum, in0=s01, in1=wc[:, 2, :],
                                        op=AL.add)
                pz = tr2.tile([128, TP], FP, tag="pz")
                nc.vector.tensor_scalar(out=pz, in0=wc[:, 0, :],
                                        scalar1=izs[:, c, 0:1], scalar2=None,
                                        op0=AL.mult)
                nc.vector.scalar_tensor_tensor(out=pz, in0=wc[:, 1, :],
                                               scalar=izs[:, c, 1:2], in1=pz,
                                               op0=AL.mult, op1=AL.add)
                nc.vector.scalar_tensor_tensor(out=pz, in0=wc[:, 2, :],
                                               scalar=izs[:, c, 2:3], in1=pz,
                                               op0=AL.mult, op1=AL.add)
                rs = keepf.tile([128, TP], FP, tag="rs", name=f"rs{c}")
                nc.vector.reciprocal_approx_fast(out=rs, in_=ssum)
                rp = tr2.tile([128, TP], FP, tag="rp")
                nc.vector.reciprocal_approx_fast(out=rp, in_=pz)
                zp = keepf.tile([128, TP], FP, tag="zp", name=f"zp{c}")
                nc.gpsimd.tensor_tensor(out=zp, in0=ssum, in1=rp, op=AL.mult)
                wc_all[c], rs_all[c], zp_all[c] = wc, rs, zp

        emit_sqrt(NC - 1)
        # gate: all P2 exp ops depend on the last chunk's ds (zero bias)
        b_gate = const.tile([128, 1], FP)
        nc.vector.tensor_scalar(out=b_gate, in0=ds_all[NC - 1][:, 0:1],
                                scalar1=0.0, scalar2=0.0,
                                op0=AL.mult, op1=AL.add)

        # ------------- P2: softplus = ln(1+exp(.)) (ln_exp table) ----------
        nmm = NC + NF * 3
        mmi = 0
        for c in range(NC):
            full = c < NF
            t = tr2.tile([128, TP], FP, tag="t")
            nc.scalar.activation(t, ds_all[c], AF.Exp, scale=1.0 / SIGMA,
                                 bias=b_gate)
            l = keep.tile([128, TP], FP if full else BF, tag="l",
                          name=f"l{c}")
            nc.scalar.activation(l, t, AF.Ln, bias=b_one)
            if full:
                lb = tr2.tile([128, TP], BF, tag="lb")
                nc.scalar.activation(lb, l, AF.Copy)
                nc.tensor.matmul(acc[:, :], ln_st[:, c, :], lb,
                                 start=(mmi == 0), stop=(mmi == nmm - 1))
            else:
                nc.tensor.matmul(acc[:, :], ln_st[:, c, :], l,
                                 start=(mmi == 0), stop=(mmi == nmm - 1))
            mmi += 1
            if full:
                l_full[c] = l

        # ------------- P3: full-population rgb/dsum accumulation ----------
        for c in range(NF):
            u1 = tr2.tile([128, TP], FP, tag="u1")
            nc.vector.scalar_tensor_tensor(out=u1, in0=ds_all[c],
                                           scalar=1.0 / SIGMA, in1=l_full[c],
                                           op0=AL.mult, op1=AL.subtract)
            u2 = tr2.tile([128, TP], FP, tag="u2")
            nc.vector.scalar_tensor_tensor(out=u2, in0=zp_all[c],
                                           scalar=zsc, in1=u1,
                                           op0=AL.mult, op1=AL.add)
            pe = tr2.tile([128, TP], FP, tag="pe")
            nc.scalar.activation(pe, u2, AF.Exp, bias=b_pe)
            t0 = tr2.tile([128, TP], FP, tag="t0")
            nc.gpsimd.tensor_tensor(out=t0, in0=pe, in1=rs_all[c], op=AL.mult)
            for k in range(3):
                g = tr2.tile([128, TP], BF, tag="g", name=f"g{k}")
                nc.vector.tensor_tensor(out=g, in0=t0, in1=wc_all[c][:, k, :],
                                        op=AL.mult)
                nc.tensor.matmul(acc[:, :], g_st[:, c, k, :], g,
                                 start=(mmi == 0), stop=(mmi == nmm - 1))
                mmi += 1

        # ---------------- P4: write out --------------------------------
        o = tr2.tile([ACC, TP], FP, tag="o")
        nc.scalar.activation(o, acc, AF.Copy)
        nc.sync.dma_start(out=d_out[:, :], in_=o)

    nc.compile()
    return nc


def kernel(face_vertices, face_textures):
    prep = _host_prep(face_vertices)
    mhat = float(max(prep['znUB'].max(), EPS))
    Tt, D = _tile_thresholds(prep, mhat)
    kf, ka, assign, NF, NA = _populate(prep, Tt, D, mhat)

    in_maps = []
    slots_all = []
    for c in range(NCORES):
        slots = _pack_slots(assign[c], kf, ka, NF, NA)
        slots_all.append(slots)
        in_maps.append(_build_inputs(prep, face_textures, slots, assign[c],
                                     NF, NA))

    nc = _build_program(NF, NA, mhat)
    global LAST_RESULT
    res = run_bass_kernel_spmd(nc, in_maps, core_ids=list(range(NCORES)),
                               trace=TRACE)
    LAST_RESULT = res

    out = np.zeros((1, 4, H, W), np.float32)
    wbg = np.exp((EPS - mhat) / GAMMA)
    for c in range(NCORES):
        o = np.asarray(res.results[c]["out"], np.float64)   # [80, TP]
        for tl, t in enumerate(assign[c]):
            ty, tx = t // NTX, t % NTX
            ys = slice(ty * TR, ty * TR + TR)
            xs = slice(tx * TC, tx * TC + TC)
            blk = o[5 * tl:5 * tl + 5]
            dsum = blk[3] + wbg
            rgb = blk[0:3] / np.maximum(dsum, 1e-37)[None]
            alpha = 1.0 - np.exp(-blk[4])
            out[0, 0:3, ys, xs] = rgb.reshape(3, TR, TC)
            out[0, 3, ys, xs] = alpha.reshape(TR, TC)
    return out


# revision 16
# speedup vs baseline: 2.3448x; 2.3448x over previous
"""SoftRas-style soft rasterizer on 8 Trainium2 NeuronCores — v2.

Structure (vs the v1 baseline, ~3.7x fewer face-pixel ops):
- Pixel tiles are 16x32 px (128 tiles); faces from *different* tiles are
  packed into shared 128-row chunks by expressing each face's affine
  coefficients in its tile's local pixel frame (basis [1, lx, ly] is
  bf16-exact).  This removes the pad-to-128 quantization of v1.
- Two populations: most (face,tile) pairs only matter for the alpha
  channel (coverage product) and run a short chain: signed distance ->
  softplus.  Only faces within a per-tile adaptive score threshold of the
  pixel-depth softmax peak run the full barycentric/depth/RGB chain.
- Per-tile accumulators (rgb x3, dsum, ln-alpha) live in one [80, 512]
  PSUM bank, fed by block-diagonal stationary matmuls.
- softplus(a) = Ln(1 + Exp(a)) keeps every Act LUT op in one table set
  (natural_log_exp); sigmoid is eliminated via prob*E = exp(a + l + zarg).
- Sign of the distance: S = max_e sLD_e via a transposed-AP tensor_reduce,
  turned into +-1.0f with one tensor_scalar bit-trick, applied by a Pool
  multiply.
"""
import sys
sys.path.insert(0, '/opt/trn_rl_repo')
import numpy as np
import ml_dtypes
from contextlib import ExitStack

import concourse.bass as bass
import concourse.bacc as bacc
import concourse.tile as tile
import concourse.mybir as mybir
from concourse.bass_utils import run_bass_kernel_spmd

TRACE = False
LAST_RESULT = None

F_TOT = 512
H = W = 256
NCORES = 8
TR, TC = 16, 32                  # tile rows/cols
TP = TR * TC                     # 512 px per tile
NTY, NTX = H // TR, W // TC      # 16 x 8 tiles
NT = NTY * NTX                   # 128 tiles
TPC = NT // NCORES               # 16 tiles per core
SIGMA = 1e-2
GAMMA = 1e-3
EPS = 1e-3
NEAR, FAR = 1.0, 100.0
GAP_A = 0.095                    # alpha keep radius (p ~ 7.5e-5)
MARGIN = 14.0                    # rgb score margin (e^-14 ~ 8e-7)

FP = mybir.dt.float32
F32R = mybir.dt.float32r
BF = mybir.dt.bfloat16
U32 = mybir.dt.uint32
AL = mybir.AluOpType
AF = mybir.ActivationFunctionType

BF16 = ml_dtypes.bfloat16


def _split2(a):
    h = np.asarray(a, np.float64).astype(BF16).astype(np.float64)
    return h, (a - h)


def _host_prep(face_vertices):
    fv = np.asarray(face_vertices, np.float64)[0]          # [F,3,3]
    F = fv.shape[0]
    x = fv[:, :, 0]; y = fv[:, :, 1]; z = fv[:, :, 2]
    x0, x1, x2 = x[:, 0], x[:, 1], x[:, 2]
    y0, y1, y2 = y[:, 0], y[:, 1], y[:, 2]

    den = (y1 - y2) * (x0 - x2) + (x2 - x1) * (y0 - y2)
    den = np.where(np.abs(den) < 1e-10, 1e-10, den)
    # barycentric coefficient rows [c0, cx, cy] per k
    Wc = np.zeros((F, 3, 3))
    Wc[:, 0] = np.stack([(-(y1 - y2) * x2 - (x2 - x1) * y2) / den,
                         (y1 - y2) / den, (x2 - x1) / den], -1)
    Wc[:, 1] = np.stack([(-(y2 - y0) * x2 - (x0 - x2) * y2) / den,
                         (y2 - y0) / den, (x0 - x2) / den], -1)
    Wc[:, 2] = -Wc[:, 0] - Wc[:, 1]
    Wc[:, 2, 0] += 1.0                                      # w2 = 1-w0-w1

    anchors = [(x0, y0), (x1, y1), (x2, y2)]
    pairs = [(0, 1), (1, 2), (2, 0)]
    # per edge: U' = ((p-a).d)/L - L/2  (along-line, centered)
    #           sLD = cross(p-a, d)/L oriented positive OUTSIDE
    Uc = np.zeros((F, 3, 3)); Lc = np.zeros((F, 3, 3)); HL = np.zeros((F, 3))
    cx = (x0 + x1 + x2) / 3.0; cy = (y0 + y1 + y2) / 3.0
    for e, (ia, ib) in enumerate(pairs):
        ax, ay = anchors[ia]; bx, by = anchors[ib]
        dx, dy = bx - ax, by - ay
        L = np.sqrt(np.maximum(dx * dx + dy * dy, 1e-12))
        iL = 1.0 / L
        Uc[:, e, 0] = (-ax * dx - ay * dy) * iL - L / 2.0
        Uc[:, e, 1] = dx * iL
        Uc[:, e, 2] = dy * iL
        sc0 = (ay * dx - ax * dy) * iL
        scx = dy * iL
        scy = -dx * iL
        v = sc0 + scx * cx + scy * cy                      # value at centroid
        orient = np.where(np.abs(v) < 1e-12, 1.0, -np.sign(v))
        Lc[:, e, 0] = sc0 * orient
        Lc[:, e, 1] = scx * orient
        Lc[:, e, 2] = scy * orient
        HL[:, e] = L / 2.0
    iz = 1.0 / z
    assert z.min() > NEAR + 0.05 and z.max() < FAR - 0.05
    # inradius bound: max inside distance -> exp(a) stays finite
    assert HL.max() < 0.6, "exp overflow guard"
    znUB = (FAR - z.min(1)) / (FAR - NEAR)
    znLB = (FAR - z.max(1)) / (FAR - NEAR)
    return dict(Wc=Wc, Uc=Uc, Lc=Lc, HL=HL, iz=iz,
                ymin=y.min(1), ymax=y.max(1), xmin=x.min(1), xmax=x.max(1),
                znUB=znUB, znLB=znLB,
                x=x, y=y, z=z, den=den)


def _tile_thresholds(prep, mhat):
    """Per-tile rgb keep-threshold: max over tile pixels of the min face
    score, + margin.  Scores evaluated exactly for the near-depth subset."""
    D = prep['znLB'].max()
    sub = np.nonzero(prep['znUB'] >= mhat - 0.1)[0]
    assert len(sub) > 0
    pix = ((np.arange(H, dtype=np.float64) + 0.5) / H) * 2.0 - 1.0
    px = pix[None, None, :]; py = pix[None, :, None]
    x = prep['x'][sub]; y = prep['y'][sub]; z = prep['z'][sub]

    best = np.full((H, W), 1e30)
    for s in range(0, len(sub), 64):
        sl = slice(s, min(s + 64, len(sub)))
        X0, X1, X2 = (x[sl, k][:, None, None] for k in range(3))
        Y0, Y1, Y2 = (y[sl, k][:, None, None] for k in range(3))
        den = (y[sl, 1] - y[sl, 2]) * (x[sl, 0] - x[sl, 2]) + \
              (x[sl, 2] - x[sl, 1]) * (y[sl, 0] - y[sl, 2])
        den = np.where(np.abs(den) < 1e-10, 1e-10, den)[:, None, None]
        w0 = ((Y1 - Y2) * (px - X2) + (X2 - X1) * (py - Y2)) / den
        w1 = ((Y2 - Y0) * (px - X2) + (X0 - X2) * (py - Y2)) / den
        w2 = 1.0 - w0 - w1
        inside = (w0 >= 0) & (w1 >= 0) & (w2 >= 0)

        def ed2(ax, ay, bx, by):
            dx, dy = bx - ax, by - ay
            l2 = np.maximum(dx * dx + dy * dy, 1e-12)
            t = np.clip(((px - ax) * dx + (py - ay) * dy) / l2, 0.0, 1.0)
            return (px - (ax + t * dx)) ** 2 + (py - (ay + t * dy)) ** 2

        d2 = np.minimum(np.minimum(ed2(X0, Y0, X1, Y1), ed2(X1, Y1, X2, Y2)),
                        ed2(X2, Y2, X0, Y0))
        dist = np.sqrt(d2 + 1e-12)
        wc0, wc1, wc2 = np.clip(w0, 0, 1), np.clip(w1, 0, 1), np.clip(w2, 0, 1)
        ssum = np.maximum(wc0 + wc1 + wc2, 1e-12)
        izl = 1.0 / z[sl]
        P = (wc0 * izl[:, 0, None, None] + wc1 * izl[:, 1, None, None]
             + wc2 * izl[:, 2, None, None]) / ssum
        zp = 1.0 / np.maximum(P, 1e-12)
        zn = np.clip((FAR - zp) / (FAR - NEAR), 0, 1)
        a = np.where(inside, dist, -dist) / SIGMA
        score = np.logaddexp(0.0, -a) + (mhat - zn) / GAMMA
        best = np.minimum(best, score.min(axis=0))
    Sstar = best.max()
    assert Sstar < 80.0, f"fp32 dsum underflow risk: S*={Sstar}"
    tb = best.reshape(NTY, TR, NTX, TC).max(axis=(1, 3))    # [NTY, NTX]
    return tb + MARGIN, D


def _populate(prep, Tt, D, mhat):
    """FULL / ALPHA face lists per tile + core assignment + chunk pattern."""
    pix = ((np.arange(H, dtype=np.float64) + 0.5) / H) * 2.0 - 1.0
    y0v = pix[np.arange(NTY) * TR]; y1v = pix[np.arange(NTY) * TR + TR - 1]
    x0v = pix[np.arange(NTX) * TC]; x1v = pix[np.arange(NTX) * TC + TC - 1]
    ygap = np.maximum(0.0, np.maximum(prep['ymin'][None] - y1v[:, None],
                                      y0v[:, None] - prep['ymax'][None]))
    xgap = np.maximum(0.0, np.maximum(prep['xmin'][None] - x1v[:, None],
                                      x0v[:, None] - prep['xmax'][None]))
    gap = np.sqrt(ygap[:, None, :] ** 2 + xgap[None, :, :] ** 2)  # [NTY,NTX,F]
    sbound = gap / SIGMA + (D - prep['znUB'])[None, None] / GAMMA
    keep_full = sbound < Tt[:, :, None]
    keep_alpha = (gap < GAP_A) & ~keep_full
    nf = keep_full.sum(2).ravel(); na = keep_alpha.sum(2).ravel()

    # assign 16 tiles/core: balance full pairs (hard-ish) then alpha
    order = np.argsort(-(nf * 4 + na), kind='stable')
    loads_f = np.zeros(NCORES, np.int64)
    loads_a = np.zeros(NCORES, np.int64)
    counts = np.zeros(NCORES, np.int64)
    assign = [[] for _ in range(NCORES)]
    for t in order:
        cands = [c for c in range(NCORES) if counts[c] < TPC]
        c = min(cands, key=lambda c: (loads_f[c] * 4 + loads_a[c]))
        assign[c].append(int(t))
        loads_f[c] += nf[t]; loads_a[c] += na[t]; counts[c] += 1
    NF = int(np.ceil(loads_f.max() / 128))
    NA = int(np.ceil(loads_a.max() / 128))
    kf = keep_full.reshape(NT, F_TOT); ka = keep_alpha.reshape(NT, F_TOT)
    return kf, ka, assign, NF, NA


def _pack_slots(assign_c, kf, ka, NF, NA):
    """Per-core slot lists: [(tile_local, face)] padded with (-1,-1)."""
    fulls, alphas = [], []
    for tl, t in enumerate(assign_c):
        for f in np.nonzero(kf[t])[0]:
            fulls.append((tl, int(f)))
        for f in np.nonzero(ka[t])[0]:
            alphas.append((tl, int(f)))
    assert len(fulls) <= NF * 128 and len(alphas) <= NA * 128
    fulls += [(-1, -1)] * (NF * 128 - len(fulls))
    alphas += [(-1, -1)] * (NA * 128 - len(alphas))
    return np.array(fulls + alphas, np.int64).reshape(NF + NA, 128, 2)


def _build_inputs(prep, textures, slots, assign_c, NF, NA):
    """All dram input arrays for one core."""
    NC = NF + NA
    pix = ((np.arange(H, dtype=np.float64) + 0.5) / H) * 2.0 - 1.0
    tl = slots[:, :, 0]; fi = slots[:, :, 1]            # [NC,128]
    dummy = fi < 0
    tlz = np.where(dummy, 0, tl); fiz = np.where(dummy, 0, fi)
    tg = np.array(assign_c)[tlz]                        # global tile id
    ty, tx = tg // NTX, tg % NTX
    ox = pix[tx * TC]; oy = pix[ty * TR]                # [NC,128]

    def localize(C):                                    # C: [F,3,3] k/e-major
        c = C[fiz]                                      # [NC,128,3,3]
        c0 = c[..., 0] + c[..., 1] * ox[..., None] + c[..., 2] * oy[..., None]
        return np.stack([c0, c[..., 1], c[..., 2]], -1)  # [NC,128,3,3]

    ld = localize(prep['Lc'])
    u = localize(prep['Uc'])
    ld[dummy] = 0.0; u[dummy] = 0.0
    ld[dummy, :, 0] = 10.0                              # far outside
    h = prep['HL'][fiz]; h[dummy] = 0.5                 # [NC,128,3]

    ldh, ldl = _split2(ld); uh, ul = _split2(u)
    st_ld = np.concatenate([ldh, ldl], axis=-1)         # [NC,128,3,6]
    st_u = np.concatenate([uh, ul], axis=-1)
    st_ld = st_ld.transpose(0, 3, 2, 1)                 # [NC,6,3,128]
    st_u = st_u.transpose(0, 3, 2, 1)

    w = localize(prep['Wc'])[:NF] if NF else np.zeros((0, 128, 3, 3))
    dw = dummy[:NF]
    w[dw] = 0.0
    w[dw, 0, 0] = -1.0; w[dw, 1, 0] = -1.0; w[dw, 2, 0] = 3.0
    w1h, r = _split2(w); w2h, w3h = _split2(r)
    st_w = np.concatenate([w1h, w2h, w3h], axis=-1)     # [NF,128,3,9]
    st_w = st_w.transpose(0, 3, 2, 1)                   # [NF,9,3,128]

    izf = prep['iz'][fiz[:NF]] if NF else np.zeros((0, 128, 3))
    izf[dummy[:NF]] = 0.011

    # accumulator stationaries: acc row layout per local tile: 5*tl + {0..4}
    ln_st = np.zeros((NC, 128, 5 * TPC))
    ln_st[np.arange(NC)[:, None], np.arange(128)[None], 5 * tlz + 4] = \
        (~dummy).astype(np.float64)

    g_st = np.zeros((NF, 3, 128, 5 * TPC))
    if NF:
        tex = np.asarray(textures, np.float64)[0][fiz[:NF]]   # [NF,128,3,3]
        tex[dummy[:NF]] = 0.0
        ar = np.arange(128)[None]
        for k in range(3):
            for c in range(3):
                g_st[np.arange(NF)[:, None], k, ar, 5 * tlz[:NF] + c] = \
                    tex[:, :, k, c]
            g_st[np.arange(NF)[:, None], k, ar, 5 * tlz[:NF] + 3] = \
                (~dummy[:NF]).astype(np.float64)

    # basis (shared, exact in bf16)
    lx = (np.arange(TP) % TC) / 128.0
    ly = (np.arange(TP) // TC) / 128.0
    b3 = np.stack([np.ones(TP), lx, ly])
    bas2 = np.concatenate([b3, b3])                     # [6,TP]
    bas3 = np.concatenate([b3, b3, b3])                 # [9,TP]

    hs = np.concatenate([h, -h], axis=-1)               # [NC,128,6]
    return {
        "st_ld": st_ld.transpose(1, 0, 2, 3).reshape(6, NC * 3 * 128)
                 .astype(BF16),
        "st_u": st_u.transpose(1, 0, 2, 3).reshape(6, NC * 3 * 128)
                .astype(BF16),
        "st_w": st_w.transpose(1, 0, 2, 3).reshape(9, max(NF, 1) * 3 * 128)
                .astype(BF16) if NF else np.zeros((9, 384), BF16),
        "scal": hs.transpose(1, 0, 2).reshape(128, NC * 6).astype(np.float32),
        "izs": izf.transpose(1, 0, 2).reshape(128, max(NF, 1) * 3)
               .astype(np.float32) if NF else np.zeros((128, 3), np.float32),
        "ln_st": ln_st.transpose(1, 0, 2).reshape(128, NC * 5 * TPC)
                 .astype(BF16),
        "g_st": g_st.transpose(2, 0, 1, 3).reshape(128, max(NF, 1) * 3 * 5 * TPC)
                .astype(BF16) if NF else np.zeros((128, 240), BF16),
        "bas2": bas2.astype(BF16),
        "bas3": bas3.astype(BF16),
    }


def _steer_act_tables():
    import functools
    import concourse.hw_specs as hw_specs
    real = hw_specs.get_activation_tables
    if getattr(real, '_steered', False):
        return
    base = real.__wrapped__ if hasattr(real, '__wrapped__') else real

    @functools.cache
    def steered(arch):
        t = dict(base(arch))
        for name in ('exp_and_others', 'natural_log', 'exp_and_friends'):
            if name in t and 'natural_log_exp_and_others' in t:
                t[name] = set()
        return t

    steered._steered = True
    hw_specs.get_activation_tables = steered
    bacc.get_activation_tables = steered


def _build_program(NF, NA, mhat):
    _steer_act_tables()
    NC = NF + NA
    ACC = 5 * TPC                                        # 80
    nc = bacc.Bacc("TRN2", target_bir_lowering=False, debug=False,
                   num_devices=NCORES)
    d_stld = nc.dram_tensor("st_ld", [6, NC * 3 * 128], BF,
                            kind="ExternalInput")
    d_stu = nc.dram_tensor("st_u", [6, NC * 3 * 128], BF,
                           kind="ExternalInput")
    d_stw = nc.dram_tensor("st_w", [9, max(NF, 1) * 3 * 128], BF,
                           kind="ExternalInput")
    d_scal = nc.dram_tensor("scal", [128, NC * 6], FP, kind="ExternalInput")
    d_izs = nc.dram_tensor("izs", [128, max(NF, 1) * 3], FP,
                           kind="ExternalInput")
    d_lnst = nc.dram_tensor("ln_st", [128, NC * ACC], BF,
                            kind="ExternalInput")
    d_gst = nc.dram_tensor("g_st", [128, max(NF, 1) * 3 * ACC], BF,
                           kind="ExternalInput")
    d_bas2 = nc.dram_tensor("bas2", [6, TP], BF, kind="ExternalInput")
    d_bas3 = nc.dram_tensor("bas3", [9, TP], BF, kind="ExternalInput")
    d_out = nc.dram_tensor("out", [ACC, TP], FP, kind="ExternalOutput")

    bias_pe = float(FAR / (FAR - NEAR) / GAMMA - 1000.0 * mhat)
    zsc = float(-1.0 / ((FAR - NEAR) * GAMMA))

    with ExitStack() as ctx:
        tc = ctx.enter_context(tile.TileContext(nc))
        const = ctx.enter_context(tc.tile_pool(name="const", bufs=1))
        tr1 = ctx.enter_context(tc.tile_pool(name="tr1", bufs=3))
        tr2 = ctx.enter_context(tc.tile_pool(name="tr2", bufs=3))
        fpool = ctx.enter_context(tc.tile_pool(name="fpool", bufs=1))
        keep = ctx.enter_context(tc.tile_pool(name="keep", bufs=NC))
        keepf = ctx.enter_context(tc.tile_pool(name="keepf", bufs=max(NF, 1)))
        qp = ctx.enter_context(tc.tile_pool(name="qp", bufs=2, space="PSUM"))
        accp = ctx.enter_context(tc.tile_pool(name="accp", bufs=1, space="PSUM"))

        st_ld = const.tile([6, NC, 3, 128], BF)
        nc.sync.dma_start(out=st_ld.rearrange("k c e f -> k (c e f)"),
                          in_=d_stld[:, :])
        st_u = const.tile([6, NC, 3, 128], BF)
        nc.sync.dma_start(out=st_u.rearrange("k c e f -> k (c e f)"),
                          in_=d_stu[:, :])
        st_w = const.tile([9, max(NF, 1), 3, 128], BF)
        nc.sync.dma_start(out=st_w.rearrange("k c e f -> k (c e f)"),
                          in_=d_stw[:, :])
        scal = const.tile([128, NC, 6], FP)
        nc.sync.dma_start(out=scal.rearrange("f c s -> f (c s)"),
                          in_=d_scal[:, :])
        izs = const.tile([128, max(NF, 1), 3], FP)
        nc.sync.dma_start(out=izs.rearrange("f c s -> f (c s)"),
                          in_=d_izs[:, :])
        ln_st = const.tile([128, NC, ACC], BF)
        nc.sync.dma_start(out=ln_st.rearrange("f c a -> f (c a)"),
                          in_=d_lnst[:, :])
        g_st = const.tile([128, max(NF, 1), 3, ACC], BF)
        nc.sync.dma_start(out=g_st.rearrange("f c k a -> f (c k a)"),
                          in_=d_gst[:, :])
        bas2 = const.tile([6, TP], BF)
        nc.sync.dma_start(out=bas2, in_=d_bas2[:, :])
        bas3 = const.tile([9, TP], BF)
        nc.sync.dma_start(out=bas3, in_=d_bas3[:, :])

        b_eps = const.tile([128, 1], FP)
        nc.vector.memset(b_eps, 1e-12)
        b_one = const.tile([128, 1], FP)
        nc.vector.memset(b_one, 1.0)
        b_pe = const.tile([128, 1], FP)
        nc.vector.memset(b_pe, bias_pe)

        acc = accp.tile([ACC, TP], FP, tag="acc")
        ds_all, wc_all, rs_all, zp_all, l_full = {}, {}, {}, {}, {}
        m2_all, m_all = {}, {}

        def emit_sqrt(c):
            d = tr2.tile([128, TP], FP, tag="d")
            nc.scalar.activation(d, m2_all[c], AF.Sqrt, bias=b_eps)
            ds = keep.tile([128, TP], FP, tag="ds", name=f"ds{c}")
            nc.gpsimd.tensor_tensor(out=ds, in0=d, in1=m_all[c], op=AL.mult)
            ds_all[c] = ds

        # ---------------- P1: distance chains (sqrt table) ----------------
        for c in range(NC):
            if c > 0:
                emit_sqrt(c - 1)
            full = c < NF
            ld = qp.tile([128, 3, TP], FP, tag="q", name=f"ld{c}")
            for e in range(3):
                for h in range(TP // 512):
                    nc.tensor.matmul(ld[:, e, 512 * h:512 * (h + 1)],
                                     st_ld[:, c, e, :],
                                     bas2[:, 512 * h:512 * (h + 1)],
                                     start=True, stop=True)
            S = tr1.tile([128, TP], FP, tag="S")
            nc.vector.tensor_reduce(out=S, in_=ld[:, :, :].transpose([0, 2, 1]),
                                    axis=mybir.AxisListType.X, op=AL.max)
            m = keep.tile([128, TP], FP, tag="m", name=f"m_{c}")
            nc.vector.tensor_scalar(out=m.bitcast(U32), in0=S.bitcast(U32),
                                    scalar1=0x80000000, scalar2=0xBF800000,
                                    op0=AL.bitwise_and, op1=AL.bitwise_xor)
            sq = tr1.tile([128, 3, TP], BF, tag="sq")
            nc.scalar.activation(sq.rearrange("f e n -> f (e n)"),
                                 ld.rearrange("f e n -> f (e n)"), AF.Square)
            u = qp.tile([128, 3, TP], FP, tag="q", name=f"u{c}")
            for e in range(3):
                for h in range(TP // 512):
                    nc.tensor.matmul(u[:, e, 512 * h:512 * (h + 1)],
                                     st_u[:, c, e, :],
                                     bas2[:, 512 * h:512 * (h + 1)],
                                     start=True, stop=True)
            aU = tr1.tile([128, 3, TP], BF, tag="aU")
            nc.scalar.activation(aU.rearrange("f e n -> f (e n)"),
                                 u.rearrange("f e n -> f (e n)"), AF.Abs)
            rm = tr1.tile([128, 3, TP], BF, tag="rm")
            for e in range(3):
                nc.vector.tensor_scalar(out=rm[:, e, :], in0=aU[:, e, :],
                                        scalar1=scal[:, c, e:e + 1],
                                        scalar2=scal[:, c, 3 + e:4 + e],
                                        op0=AL.max, op1=AL.add)
            rsq = tr1.tile([128, 3, TP], BF, tag="rsq")
            nc.vector.tensor_tensor(out=rsq.rearrange("f e n -> f (e n)"),
                                    in0=rm.rearrange("f e n -> f (e n)"),
                                    in1=rm.rearrange("f e n -> f (e n)"),
                                    op=AL.mult)
            a3 = tr1.tile([128, 3, TP], BF, tag="a3")
            nc.vector.tensor_tensor(out=a3.rearrange("f e n -> f (e n)"),
                                    in0=rsq.rearrange("f e n -> f (e n)"),
                                    in1=sq.rearrange("f e n -> f (e n)"),
                                    op=AL.add)
            m1 = tr2.tile([128, TP], BF, tag="m1")
            nc.vector.tensor_tensor(out=m1, in0=a3[:, 0, :], in1=a3[:, 1, :],
                                    op=AL.min)
            m2 = keep.tile([128, TP], BF, tag="m2", name=f"m2_{c}")
            nc.vector.tensor_tensor(out=m2, in0=m1, in1=a3[:, 2, :],
                                    op=AL.min)
            m2_all[c], m_all[c] = m2, m

            if full:
                wq = qp.tile([128, 3, TP], FP, tag="q", name=f"w{c}")
                for k in range(3):
                    for h in range(TP // 512):
                        nc.tensor.matmul(wq[:, k, 512 * h:512 * (h + 1)],
                                         st_w[:, c, k, :],
                                         bas3[:, 512 * h:512 * (h + 1)],
                                         start=True, stop=True)
                wc = keepf.tile([128, 3, TP], FP, tag="wc", name=f"wc{c}")
                for k in range(3):
                    nc.vector.tensor_scalar(out=wc[:, k, :], in0=wq[:, k, :],
                                            scalar1=0.0, scalar2=1.0,
                                            op0=AL.max, op1=AL.min)
                s01 = fpool.tile([128, TP], FP, tag="s01")
                nc.gpsimd.tensor_tensor(out=s01, in0=wc[:, 0, :],
                                        in1=wc[:, 1, :], op=AL.add)
                ssum = fpool.tile([128, TP], FP, tag="ssum")
                nc.gpsimd.tensor_tensor(out=ssum, in0=s01, in1=wc[:, 2, :],
                                        op=AL.add)
                pz = fpool.tile([128, TP], FP, tag="pz")
                nc.vector.tensor_scalar(out=pz, in0=wc[:, 0, :],
                                        scalar1=izs[:, c, 0:1], scalar2=None,
                                        op0=AL.mult)
                nc.vector.scalar_tensor_tensor(out=pz, in0=wc[:, 1, :],
                                               scalar=izs[:, c, 1:2], in1=pz,
                                               op0=AL.mult, op1=AL.add)
                nc.vector.scalar_tensor_tensor(out=pz, in0=wc[:, 2, :],
                                               scalar=izs[:, c, 2:3], in1=pz,
                                               op0=AL.mult, op1=AL.add)
                rs = keepf.tile([128, TP], FP, tag="rs", name=f"rs{c}")
                nc.vector.reciprocal_approx_fast(out=rs, in_=ssum)
                rp = fpool.tile([128, TP], FP, tag="rp")
                nc.vector.reciprocal_approx_fast(out=rp, in_=pz)
                zp = keepf.tile([128, TP], FP, tag="zp", name=f"zp{c}")
                nc.gpsimd.tensor_tensor(out=zp, in0=ssum, in1=rp, op=AL.mult)
                wc_all[c], rs_all[c], zp_all[c] = wc, rs, zp

        emit_sqrt(NC - 1)
        # gate: all P2 exp ops depend on the last chunk's ds (zero bias)
        b_gate = const.tile([128, 1], FP)
        nc.vector.tensor_scalar(out=b_gate, in0=ds_all[NC - 1][:, 0:1],
                                scalar1=0.0, scalar2=0.0,
                                op0=AL.mult, op1=AL.add)

        # ------------- P2: softplus = ln(1+exp(.)) (ln_exp table) ----------
        nmm = (NC + NF * 3) * (TP // 512)
        mmi = 0
        for c in range(NC):
            full = c < NF
            t = tr2.tile([128, TP], FP, tag="t")
            nc.scalar.activation(t, ds_all[c], AF.Exp, scale=1.0 / SIGMA,
                                 bias=b_gate)
            l = keep.tile([128, TP], FP if full else BF, tag="l",
                          name=f"l{c}")
            nc.scalar.activation(l, t, AF.Ln, bias=b_one)
            if full:
                lb = tr2.tile([128, TP], BF, tag="lb")
                nc.scalar.activation(lb, l, AF.Copy)
                mv = lb
            else:
                mv = l
            for h in range(TP // 512):
                nc.tensor.matmul(acc[:, 512 * h:512 * (h + 1)],
                                 ln_st[:, c, :], mv[:, 512 * h:512 * (h + 1)],
                                 start=(mmi == 0), stop=(mmi == nmm - 1))
                mmi += 1
            if full:
                l_full[c] = l

        # ------------- P3: full-population rgb/dsum accumulation ----------
        for c in range(NF):
            u1 = fpool.tile([128, TP], FP, tag="u1")
            nc.vector.scalar_tensor_tensor(out=u1, in0=ds_all[c],
                                           scalar=1.0 / SIGMA, in1=l_full[c],
                                           op0=AL.mult, op1=AL.subtract)
            u2 = fpool.tile([128, TP], FP, tag="u2")
            nc.vector.scalar_tensor_tensor(out=u2, in0=zp_all[c],
                                           scalar=zsc, in1=u1,
                                           op0=AL.mult, op1=AL.add)
            pe = fpool.tile([128, TP], FP, tag="pe")
            nc.scalar.activation(pe, u2, AF.Exp, bias=b_pe)
            t0 = fpool.tile([128, TP], FP, tag="t0")
            nc.gpsimd.tensor_tensor(out=t0, in0=pe, in1=rs_all[c], op=AL.mult)
            for k in range(3):
                g = tr2.tile([128, TP], BF, tag="g", name=f"g{k}")
                nc.vector.tensor_tensor(out=g, in0=t0, in1=wc_all[c][:, k, :],
                                        op=AL.mult)
                for h in range(TP // 512):
                    nc.tensor.matmul(acc[:, 512 * h:512 * (h + 1)],
                                     g_st[:, c, k, :],
                                     g[:, 512 * h:512 * (h + 1)],
                                     start=(mmi == 0), stop=(mmi == nmm - 1))
                    mmi += 1

        # ---------------- P4: write out --------------------------------
        o = fpool.tile([ACC, TP], FP, tag="o")
        nc.scalar.activation(o, acc, AF.Copy)
        nc.sync.dma_start(out=d_out[:, :], in_=o)

    nc.compile()
    return nc


def kernel(face_vertices, face_textures):
    prep = _host_prep(face_vertices)
    mhat = float(max(prep['znUB'].max(), EPS))
    Tt, D = _tile_thresholds(prep, mhat)
    kf, ka, assign, NF, NA = _populate(prep, Tt, D, mhat)

    in_maps = []
    slots_all = []
    for c in range(NCORES):
        slots = _pack_slots(assign[c], kf, ka, NF, NA)
        slots_all.append(slots)
        in_maps.append(_build_inputs(prep, face_textures, slots, assign[c],
                                     NF, NA))

    nc = _build_program(NF, NA, mhat)
    global LAST_RESULT
    res = run_bass_kernel_spmd(nc, in_maps, core_ids=list(range(NCORES)),
                               trace=TRACE)
    LAST_RESULT = res

    out = np.zeros((1, 4, H, W), np.float32)
    wbg = np.exp((EPS - mhat) / GAMMA)
    for c in range(NCORES):
        o = np.asarray(res.results[c]["out"], np.float64)   # [80, TP]
        for tl, t in enumerate(assign[c]):
            ty, tx = t // NTX, t % NTX
            ys = slice(ty * TR, ty * TR + TR)
            xs = slice(tx * TC, tx * TC + TC)
            blk = o[5 * tl:5 * tl + 5]
            dsum = blk[3] + wbg
            rgb = blk[0:3] / np.maximum(dsum, 1e-37)[None]
            alpha = 1.0 - np.exp(-blk[4])
            out[0, 0:3, ys, xs] = rgb.reshape(3, TR, TC)
            out[0, 3, ys, xs] = alpha.reshape(TR, TC)
    return out


# revision 17
# speedup vs baseline: 2.4396x; 1.0404x over previous
"""SoftRas-style soft rasterizer on 8 Trainium2 NeuronCores — v2.

Structure (vs the v1 baseline, ~3.7x fewer face-pixel ops):
- Pixel tiles are 16x32 px (128 tiles); faces from *different* tiles are
  packed into shared 128-row chunks by expressing each face's affine
  coefficients in its tile's local pixel frame (basis [1, lx, ly] is
  bf16-exact).  This removes the pad-to-128 quantization of v1.
- Two populations: most (face,tile) pairs only matter for the alpha
  channel (coverage product) and run a short chain: signed distance ->
  softplus.  Only faces within a per-tile adaptive score threshold of the
  pixel-depth softmax peak run the full barycentric/depth/RGB chain.
- Per-tile accumulators (rgb x3, dsum, ln-alpha) live in one [80, 512]
  PSUM bank, fed by block-diagonal stationary matmuls.
- softplus(a) = Ln(1 + Exp(a)) keeps every Act LUT op in one table set
  (natural_log_exp); sigmoid is eliminated via prob*E = exp(a + l + zarg).
- Sign of the distance: S = max_e sLD_e via a transposed-AP tensor_reduce,
  turned into +-1.0f with one tensor_scalar bit-trick, applied by a Pool
  multiply.
"""
import sys
sys.path.insert(0, '/opt/trn_rl_repo')
import numpy as np
import ml_dtypes
from contextlib import ExitStack

import concourse.bass as bass
import concourse.bacc as bacc
import concourse.tile as tile
import concourse.mybir as mybir
from concourse.bass_utils import run_bass_kernel_spmd

TRACE = False
LAST_RESULT = None

F_TOT = 512
H = W = 256
NCORES = 8
TR, TC = 16, 32                  # tile rows/cols
TP = TR * TC                     # 512 px per tile
NTY, NTX = H // TR, W // TC      # 16 x 8 tiles
NT = NTY * NTX                   # 128 tiles
TPC = NT // NCORES               # 16 tiles per core
SIGMA = 1e-2
GAMMA = 1e-3
EPS = 1e-3
NEAR, FAR = 1.0, 100.0
GAP_A = 0.095                    # alpha keep radius (p ~ 7.5e-5)
MARGIN = 14.0                    # rgb score margin (e^-14 ~ 8e-7)

FP = mybir.dt.float32
F32R = mybir.dt.float32r
BF = mybir.dt.bfloat16
U32 = mybir.dt.uint32
AL = mybir.AluOpType
AF = mybir.ActivationFunctionType

BF16 = ml_dtypes.bfloat16


def _split2(a):
    h = np.asarray(a, np.float64).astype(BF16).astype(np.float64)
    return h, (a - h)


def _host_prep(face_vertices):
    fv = np.asarray(face_vertices, np.float64)[0]          # [F,3,3]
    F = fv.shape[0]
    x = fv[:, :, 0]; y = fv[:, :, 1]; z = fv[:, :, 2]
    x0, x1, x2 = x[:, 0], x[:, 1], x[:, 2]
    y0, y1, y2 = y[:, 0], y[:, 1], y[:, 2]

    den = (y1 - y2) * (x0 - x2) + (x2 - x1) * (y0 - y2)
    den = np.where(np.abs(den) < 1e-10, 1e-10, den)
    # barycentric coefficient rows [c0, cx, cy] per k
    Wc = np.zeros((F, 3, 3))
    Wc[:, 0] = np.stack([(-(y1 - y2) * x2 - (x2 - x1) * y2) / den,
                         (y1 - y2) / den, (x2 - x1) / den], -1)
    Wc[:, 1] = np.stack([(-(y2 - y0) * x2 - (x0 - x2) * y2) / den,
                         (y2 - y0) / den, (x0 - x2) / den], -1)
    Wc[:, 2] = -Wc[:, 0] - Wc[:, 1]
    Wc[:, 2, 0] += 1.0                                      # w2 = 1-w0-w1

    anchors = [(x0, y0), (x1, y1), (x2, y2)]
    pairs = [(0, 1), (1, 2), (2, 0)]
    # per edge: U' = ((p-a).d)/L - L/2  (along-line, centered)
    #           sLD = cross(p-a, d)/L oriented positive OUTSIDE
    Uc = np.zeros((F, 3, 3)); Lc = np.zeros((F, 3, 3)); HL = np.zeros((F, 3))
    cx = (x0 + x1 + x2) / 3.0; cy = (y0 + y1 + y2) / 3.0
    for e, (ia, ib) in enumerate(pairs):
        ax, ay = anchors[ia]; bx, by = anchors[ib]
        dx, dy = bx - ax, by - ay
        L = np.sqrt(np.maximum(dx * dx + dy * dy, 1e-12))
        iL = 1.0 / L
        Uc[:, e, 0] = (-ax * dx - ay * dy) * iL - L / 2.0
        Uc[:, e, 1] = dx * iL
        Uc[:, e, 2] = dy * iL
        sc0 = (ay * dx - ax * dy) * iL
        scx = dy * iL
        scy = -dx * iL
        v = sc0 + scx * cx + scy * cy                      # value at centroid
        orient = np.where(np.abs(v) < 1e-12, 1.0, -np.sign(v))
        Lc[:, e, 0] = sc0 * orient
        Lc[:, e, 1] = scx * orient
        Lc[:, e, 2] = scy * orient
        HL[:, e] = L / 2.0
    iz = 1.0 / z
    assert z.min() > NEAR + 0.05 and z.max() < FAR - 0.05
    # inradius bound: max inside distance -> exp(a) stays finite
    assert HL.max() < 0.6, "exp overflow guard"
    znUB = (FAR - z.min(1)) / (FAR - NEAR)
    znLB = (FAR - z.max(1)) / (FAR - NEAR)
    return dict(Wc=Wc, Uc=Uc, Lc=Lc, HL=HL, iz=iz,
                ymin=y.min(1), ymax=y.max(1), xmin=x.min(1), xmax=x.max(1),
                znUB=znUB, znLB=znLB,
                x=x, y=y, z=z, den=den)


def _tile_thresholds(prep, mhat):
    """Per-tile rgb keep-threshold: max over tile pixels of the min face
    score, + margin.  Scores evaluated exactly for the near-depth subset."""
    D = prep['znLB'].max()
    sub = np.nonzero(prep['znUB'] >= mhat - 0.1)[0]
    assert len(sub) > 0
    pix = ((np.arange(H, dtype=np.float64) + 0.5) / H) * 2.0 - 1.0
    px = pix[None, None, :]; py = pix[None, :, None]
    x = prep['x'][sub]; y = prep['y'][sub]; z = prep['z'][sub]

    best = np.full((H, W), 1e30)
    for s in range(0, len(sub), 64):
        sl = slice(s, min(s + 64, len(sub)))
        X0, X1, X2 = (x[sl, k][:, None, None] for k in range(3))
        Y0, Y1, Y2 = (y[sl, k][:, None, None] for k in range(3))
        den = (y[sl, 1] - y[sl, 2]) * (x[sl, 0] - x[sl, 2]) + \
              (x[sl, 2] - x[sl, 1]) * (y[sl, 0] - y[sl, 2])
        den = np.where(np.abs(den) < 1e-10, 1e-10, den)[:, None, None]
        w0 = ((Y1 - Y2) * (px - X2) + (X2 - X1) * (py - Y2)) / den
        w1 = ((Y2 - Y0) * (px - X2) + (X0 - X2) * (py - Y2)) / den
        w2 = 1.0 - w0 - w1
        inside = (w0 >= 0) & (w1 >= 0) & (w2 >= 0)

        def ed2(ax, ay, bx, by):
            dx, dy = bx - ax, by - ay
            l2 = np.maximum(dx * dx + dy * dy, 1e-12)
            t = np.clip(((px - ax) * dx + (py - ay) * dy) / l2, 0.0, 1.0)
            return (px - (ax + t * dx)) ** 2 + (py - (ay + t * dy)) ** 2

        d2 = np.minimum(np.minimum(ed2(X0, Y0, X1, Y1), ed2(X1, Y1, X2, Y2)),
                        ed2(X2, Y2, X0, Y0))
        dist = np.sqrt(d2 + 1e-12)
        wc0, wc1, wc2 = np.clip(w0, 0, 1), np.clip(w1, 0, 1), np.clip(w2, 0, 1)
        ssum = np.maximum(wc0 + wc1 + wc2, 1e-12)
        izl = 1.0 / z[sl]
        P = (wc0 * izl[:, 0, None, None] + wc1 * izl[:, 1, None, None]
             + wc2 * izl[:, 2, None, None]) / ssum
        zp = 1.0 / np.maximum(P, 1e-12)
        zn = np.clip((FAR - zp) / (FAR - NEAR), 0, 1)
        a = np.where(inside, dist, -dist) / SIGMA
        score = np.logaddexp(0.0, -a) + (mhat - zn) / GAMMA
        best = np.minimum(best, score.min(axis=0))
    Sstar = best.max()
    assert Sstar < 80.0, f"fp32 dsum underflow risk: S*={Sstar}"
    tb = best.reshape(NTY, TR, NTX, TC).max(axis=(1, 3))    # [NTY, NTX]
    return tb + MARGIN, D


def _populate(prep, Tt, D, mhat):
    """FULL / ALPHA face lists per tile + core assignment + chunk pattern."""
    pix = ((np.arange(H, dtype=np.float64) + 0.5) / H) * 2.0 - 1.0
    y0v = pix[np.arange(NTY) * TR]; y1v = pix[np.arange(NTY) * TR + TR - 1]
    x0v = pix[np.arange(NTX) * TC]; x1v = pix[np.arange(NTX) * TC + TC - 1]
    ygap = np.maximum(0.0, np.maximum(prep['ymin'][None] - y1v[:, None],
                                      y0v[:, None] - prep['ymax'][None]))
    xgap = np.maximum(0.0, np.maximum(prep['xmin'][None] - x1v[:, None],
                                      x0v[:, None] - prep['xmax'][None]))
    gap = np.sqrt(ygap[:, None, :] ** 2 + xgap[None, :, :] ** 2)  # [NTY,NTX,F]
    sbound = gap / SIGMA + (D - prep['znUB'])[None, None] / GAMMA
    keep_full = sbound < Tt[:, :, None]
    keep_alpha = (gap < GAP_A) & ~keep_full
    nf = keep_full.sum(2).ravel(); na = keep_alpha.sum(2).ravel()

    # assign 16 tiles/core: balance full pairs (hard-ish) then alpha
    order = np.argsort(-(nf * 4 + na), kind='stable')
    loads_f = np.zeros(NCORES, np.int64)
    loads_a = np.zeros(NCORES, np.int64)
    counts = np.zeros(NCORES, np.int64)
    assign = [[] for _ in range(NCORES)]
    for t in order:
        cands = [c for c in range(NCORES) if counts[c] < TPC]
        c = min(cands, key=lambda c: (loads_f[c] * 4 + loads_a[c]))
        assign[c].append(int(t))
        loads_f[c] += nf[t]; loads_a[c] += na[t]; counts[c] += 1
    NF = int(np.ceil(loads_f.max() / 128))
    NA = int(np.ceil(loads_a.max() / 128))
    kf = keep_full.reshape(NT, F_TOT); ka = keep_alpha.reshape(NT, F_TOT)
    return kf, ka, assign, NF, NA


def _pack_slots(assign_c, kf, ka, NF, NA):
    """Per-core slot lists: [(tile_local, face)] padded with (-1,-1)."""
    fulls, alphas = [], []
    for tl, t in enumerate(assign_c):
        for f in np.nonzero(kf[t])[0]:
            fulls.append((tl, int(f)))
        for f in np.nonzero(ka[t])[0]:
            alphas.append((tl, int(f)))
    assert len(fulls) <= NF * 128 and len(alphas) <= NA * 128
    fulls += [(-1, -1)] * (NF * 128 - len(fulls))
    alphas += [(-1, -1)] * (NA * 128 - len(alphas))
    return np.array(fulls + alphas, np.int64).reshape(NF + NA, 128, 2)


def _build_inputs(prep, textures, slots, assign_c, NF, NA):
    """All dram input arrays for one core."""
    NC = NF + NA
    pix = ((np.arange(H, dtype=np.float64) + 0.5) / H) * 2.0 - 1.0
    tl = slots[:, :, 0]; fi = slots[:, :, 1]            # [NC,128]
    dummy = fi < 0
    tlz = np.where(dummy, 0, tl); fiz = np.where(dummy, 0, fi)
    tg = np.array(assign_c)[tlz]                        # global tile id
    ty, tx = tg // NTX, tg % NTX
    ox = pix[tx * TC]; oy = pix[ty * TR]                # [NC,128]

    def localize(C):                                    # C: [F,3,3] k/e-major
        c = C[fiz]                                      # [NC,128,3,3]
        c0 = c[..., 0] + c[..., 1] * ox[..., None] + c[..., 2] * oy[..., None]
        return np.stack([c0, c[..., 1], c[..., 2]], -1)  # [NC,128,3,3]

    ld = localize(prep['Lc'])
    u = localize(prep['Uc'])
    ld[dummy] = 0.0; u[dummy] = 0.0
    ld[dummy, :, 0] = 10.0                              # far outside
    h = prep['HL'][fiz]; h[dummy] = 0.5                 # [NC,128,3]

    ldh, ldl = _split2(ld); uh, ul = _split2(u)
    st_ld = np.concatenate([ldh, ldl], axis=-1)         # [NC,128,3,6]
    st_u = np.concatenate([uh, ul], axis=-1)
    st_ld = st_ld.transpose(0, 3, 2, 1)                 # [NC,6,3,128]
    st_u = st_u.transpose(0, 3, 2, 1)

    w = localize(prep['Wc'])[:NF] if NF else np.zeros((0, 128, 3, 3))
    dw = dummy[:NF]
    w[dw] = 0.0
    w[dw, 0, 0] = -1.0; w[dw, 1, 0] = -1.0; w[dw, 2, 0] = 3.0
    w1h, r = _split2(w); w2h, w3h = _split2(r)
    st_w = np.concatenate([w1h, w2h, w3h], axis=-1)     # [NF,128,3,9]
    st_w = st_w.transpose(0, 3, 2, 1)                   # [NF,9,3,128]

    izf = prep['iz'][fiz[:NF]] if NF else np.zeros((0, 128, 3))
    izf[dummy[:NF]] = 0.011

    # accumulator stationaries: acc row layout per local tile: 5*tl + {0..4}
    ln_st = np.zeros((NC, 128, 5 * TPC))
    ln_st[np.arange(NC)[:, None], np.arange(128)[None], 5 * tlz + 4] = \
        (~dummy).astype(np.float64)

    g_st = np.zeros((NF, 3, 128, 5 * TPC))
    if NF:
        tex = np.asarray(textures, np.float64)[0][fiz[:NF]]   # [NF,128,3,3]
        tex[dummy[:NF]] = 0.0
        ar = np.arange(128)[None]
        for k in range(3):
            for c in range(3):
                g_st[np.arange(NF)[:, None], k, ar, 5 * tlz[:NF] + c] = \
                    tex[:, :, k, c]
            g_st[np.arange(NF)[:, None], k, ar, 5 * tlz[:NF] + 3] = \
                (~dummy[:NF]).astype(np.float64)

    # basis (shared, exact in bf16)
    lx = (np.arange(TP) % TC) / 128.0
    ly = (np.arange(TP) // TC) / 128.0
    b3 = np.stack([np.ones(TP), lx, ly])
    bas2 = np.concatenate([b3, b3])                     # [6,TP]
    bas3 = np.concatenate([b3, b3, b3])                 # [9,TP]

    hs = np.concatenate([h, -h], axis=-1)               # [NC,128,6]
    return {
        "st_ld": st_ld.transpose(1, 0, 2, 3).reshape(6, NC * 3 * 128)
                 .astype(BF16),
        "st_u": st_u.transpose(1, 0, 2, 3).reshape(6, NC * 3 * 128)
                .astype(BF16),
        "st_w": st_w.transpose(1, 0, 2, 3).reshape(9, max(NF, 1) * 3 * 128)
                .astype(BF16) if NF else np.zeros((9, 384), BF16),
        "scal": hs.transpose(1, 0, 2).reshape(128, NC * 6).astype(np.float32),
        "izs": izf.transpose(1, 0, 2).reshape(128, max(NF, 1) * 3)
               .astype(np.float32) if NF else np.zeros((128, 3), np.float32),
        "ln_st": ln_st.transpose(1, 0, 2).reshape(128, NC * 5 * TPC)
                 .astype(BF16),
        "g_st": g_st.transpose(2, 0, 1, 3).reshape(128, max(NF, 1) * 3 * 5 * TPC)
                .astype(BF16) if NF else np.zeros((128, 240), BF16),
        "bas2": bas2.astype(BF16),
        "bas3": bas3.astype(BF16),
    }


def _steer_act_tables():
    import functools
    import concourse.hw_specs as hw_specs
    real = hw_specs.get_activation_tables
    if getattr(real, '_steered', False):
        return
    base = real.__wrapped__ if hasattr(real, '__wrapped__') else real

    @functools.cache
    def steered(arch):
        t = dict(base(arch))
        for name in ('exp_and_others', 'natural_log', 'exp_and_friends'):
            if name in t and 'natural_log_exp_and_others' in t:
                t[name] = set()
        return t

    steered._steered = True
    hw_specs.get_activation_tables = steered
    bacc.get_activation_tables = steered


def _build_program(NF, NA, mhat):
    _steer_act_tables()
    NC = NF + NA
    ACC = 5 * TPC                                        # 80
    nc = bacc.Bacc("TRN2", target_bir_lowering=False, debug=False,
                   num_devices=NCORES)
    d_stld = nc.dram_tensor("st_ld", [6, NC * 3 * 128], BF,
                            kind="ExternalInput")
    d_stu = nc.dram_tensor("st_u", [6, NC * 3 * 128], BF,
                           kind="ExternalInput")
    d_stw = nc.dram_tensor("st_w", [9, max(NF, 1) * 3 * 128], BF,
                           kind="ExternalInput")
    d_scal = nc.dram_tensor("scal", [128, NC * 6], FP, kind="ExternalInput")
    d_izs = nc.dram_tensor("izs", [128, max(NF, 1) * 3], FP,
                           kind="ExternalInput")
    d_lnst = nc.dram_tensor("ln_st", [128, NC * ACC], BF,
                            kind="ExternalInput")
    d_gst = nc.dram_tensor("g_st", [128, max(NF, 1) * 3 * ACC], BF,
                           kind="ExternalInput")
    d_bas2 = nc.dram_tensor("bas2", [6, TP], BF, kind="ExternalInput")
    d_bas3 = nc.dram_tensor("bas3", [9, TP], BF, kind="ExternalInput")
    d_out = nc.dram_tensor("out", [ACC, TP], FP, kind="ExternalOutput")

    bias_pe = float(FAR / (FAR - NEAR) / GAMMA - 1000.0 * mhat)
    zsc = float(-1.0 / ((FAR - NEAR) * GAMMA))

    with ExitStack() as ctx:
        tc = ctx.enter_context(tile.TileContext(nc))
        const = ctx.enter_context(tc.tile_pool(name="const", bufs=1))
        tr1 = ctx.enter_context(tc.tile_pool(name="tr1", bufs=3))
        tr2 = ctx.enter_context(tc.tile_pool(name="tr2", bufs=3))
        fpool = ctx.enter_context(tc.tile_pool(name="fpool", bufs=1))
        keep = ctx.enter_context(tc.tile_pool(name="keep", bufs=NC))
        keepf = ctx.enter_context(tc.tile_pool(name="keepf", bufs=max(NF, 1)))
        qp = ctx.enter_context(tc.tile_pool(name="qp", bufs=2, space="PSUM"))
        accp = ctx.enter_context(tc.tile_pool(name="accp", bufs=1, space="PSUM"))

        st_ld = const.tile([6, NC, 3, 128], BF)
        nc.sync.dma_start(out=st_ld.rearrange("k c e f -> k (c e f)"),
                          in_=d_stld[:, :])
        st_u = const.tile([6, NC, 3, 128], BF)
        nc.sync.dma_start(out=st_u.rearrange("k c e f -> k (c e f)"),
                          in_=d_stu[:, :])
        st_w = const.tile([9, max(NF, 1), 3, 128], BF)
        nc.sync.dma_start(out=st_w.rearrange("k c e f -> k (c e f)"),
                          in_=d_stw[:, :])
        scal = const.tile([128, NC, 6], FP)
        nc.sync.dma_start(out=scal.rearrange("f c s -> f (c s)"),
                          in_=d_scal[:, :])
        izs = const.tile([128, max(NF, 1), 3], FP)
        nc.sync.dma_start(out=izs.rearrange("f c s -> f (c s)"),
                          in_=d_izs[:, :])
        ln_st = const.tile([128, NC, ACC], BF)
        nc.sync.dma_start(out=ln_st.rearrange("f c a -> f (c a)"),
                          in_=d_lnst[:, :])
        g_st = const.tile([128, max(NF, 1), 3, ACC], BF)
        nc.sync.dma_start(out=g_st.rearrange("f c k a -> f (c k a)"),
                          in_=d_gst[:, :])
        bas2 = const.tile([6, TP], BF)
        nc.sync.dma_start(out=bas2, in_=d_bas2[:, :])
        bas3 = const.tile([9, TP], BF)
        nc.sync.dma_start(out=bas3, in_=d_bas3[:, :])

        b_eps = const.tile([128, 1], FP)
        nc.vector.memset(b_eps, 1e-12)
        b_one = const.tile([128, 1], FP)
        nc.vector.memset(b_one, 1.0)
        b_pe = const.tile([128, 1], FP)
        nc.vector.memset(b_pe, bias_pe)

        acc = accp.tile([ACC, TP], FP, tag="acc")
        ds_all, wc_all, rs_all, zp_all, l_full = {}, {}, {}, {}, {}
        m2_all, m_all = {}, {}

        def emit_sqrt(c):
            d = tr2.tile([128, TP], FP, tag="d")
            nc.scalar.activation(d, m2_all[c], AF.Sqrt, bias=b_eps)
            ds = keep.tile([128, TP], FP, tag="ds", name=f"ds{c}")
            nc.gpsimd.tensor_tensor(out=ds, in0=d, in1=m_all[c], op=AL.mult)
            ds_all[c] = ds

        # ---------------- P1: distance chains (sqrt table) ----------------
        for c in range(NC):
            if c > 0:
                emit_sqrt(c - 1)
            full = c < NF
            ld = qp.tile([128, 3, TP], FP, tag="q", name=f"ld{c}")
            for e in range(3):
                for h in range(TP // 512):
                    nc.tensor.matmul(ld[:, e, 512 * h:512 * (h + 1)],
                                     st_ld[:, c, e, :],
                                     bas2[:, 512 * h:512 * (h + 1)],
                                     start=True, stop=True)
            S = tr1.tile([128, TP], FP, tag="S")
            nc.vector.tensor_reduce(out=S, in_=ld[:, :, :].transpose([0, 2, 1]),
                                    axis=mybir.AxisListType.X, op=AL.max)
            m = keep.tile([128, TP], FP, tag="m", name=f"m_{c}")
            nc.vector.tensor_scalar(out=m.bitcast(U32), in0=S.bitcast(U32),
                                    scalar1=0x80000000, scalar2=0xBF800000,
                                    op0=AL.bitwise_and, op1=AL.bitwise_xor)
            sq = tr1.tile([128, 3, TP], BF, tag="sq")
            nc.scalar.activation(sq.rearrange("f e n -> f (e n)"),
                                 ld.rearrange("f e n -> f (e n)"), AF.Square)
            u = qp.tile([128, 3, TP], FP, tag="q", name=f"u{c}")
            for e in range(3):
                for h in range(TP // 512):
                    nc.tensor.matmul(u[:, e, 512 * h:512 * (h + 1)],
                                     st_u[:, c, e, :],
                                     bas2[:, 512 * h:512 * (h + 1)],
                                     start=True, stop=True)
            aU = tr1.tile([128, 3, TP], BF, tag="aU")
            nc.scalar.activation(aU.rearrange("f e n -> f (e n)"),
                                 u.rearrange("f e n -> f (e n)"), AF.Abs)
            rm = tr1.tile([128, 3, TP], BF, tag="rm")
            for e in range(3):
                nc.vector.tensor_scalar(out=rm[:, e, :], in0=aU[:, e, :],
                                        scalar1=scal[:, c, e:e + 1],
                                        scalar2=scal[:, c, 3 + e:4 + e],
                                        op0=AL.max, op1=AL.add)
            rsq = tr1.tile([128, 3, TP], BF, tag="rsq")
            nc.vector.tensor_tensor(out=rsq.rearrange("f e n -> f (e n)"),
                                    in0=rm.rearrange("f e n -> f (e n)"),
                                    in1=rm.rearrange("f e n -> f (e n)"),
                                    op=AL.mult)
            a3 = tr1.tile([128, 3, TP], BF, tag="a3")
            nc.vector.tensor_tensor(out=a3.rearrange("f e n -> f (e n)"),
                                    in0=rsq.rearrange("f e n -> f (e n)"),
                                    in1=sq.rearrange("f e n -> f (e n)"),
                                    op=AL.add)
            m1 = tr2.tile([128, TP], BF, tag="m1")
            nc.vector.tensor_tensor(out=m1, in0=a3[:, 0, :], in1=a3[:, 1, :],
                                    op=AL.min)
            m2 = keep.tile([128, TP], BF, tag="m2", name=f"m2_{c}")
            nc.vector.tensor_tensor(out=m2, in0=m1, in1=a3[:, 2, :],
                                    op=AL.min)
            m2_all[c], m_all[c] = m2, m

            if full:
                wq = qp.tile([128, 3, TP], FP, tag="q", name=f"w{c}")
                for k in range(3):
                    for h in range(TP // 512):
                        nc.tensor.matmul(wq[:, k, 512 * h:512 * (h + 1)],
                                         st_w[:, c, k, :],
                                         bas3[:, 512 * h:512 * (h + 1)],
                                         start=True, stop=True)
                wc = keepf.tile([128, 3, TP], FP, tag="wc", name=f"wc{c}")
                for k in range(3):
                    nc.vector.tensor_scalar(out=wc[:, k, :], in0=wq[:, k, :],
                                            scalar1=0.0, scalar2=1.0,
                                            op0=AL.max, op1=AL.min)
                s01 = fpool.tile([128, TP], FP, tag="s01")
                nc.gpsimd.tensor_tensor(out=s01, in0=wc[:, 0, :],
                                        in1=wc[:, 1, :], op=AL.add)
                ssum = fpool.tile([128, TP], FP, tag="ssum")
                nc.gpsimd.tensor_tensor(out=ssum, in0=s01, in1=wc[:, 2, :],
                                        op=AL.add)
                pz = fpool.tile([128, TP], FP, tag="pz")
                nc.vector.tensor_scalar(out=pz, in0=wc[:, 0, :],
                                        scalar1=izs[:, c, 0:1], scalar2=None,
                                        op0=AL.mult)
                nc.vector.scalar_tensor_tensor(out=pz, in0=wc[:, 1, :],
                                               scalar=izs[:, c, 1:2], in1=pz,
                                               op0=AL.mult, op1=AL.add)
                nc.vector.scalar_tensor_tensor(out=pz, in0=wc[:, 2, :],
                                               scalar=izs[:, c, 2:3], in1=pz,
                                               op0=AL.mult, op1=AL.add)
                rs = keepf.tile([128, TP], FP, tag="rs", name=f"rs{c}")
                nc.vector.reciprocal_approx_fast(out=rs, in_=ssum)
                rp = fpool.tile([128, TP], FP, tag="rp")
                nc.vector.reciprocal_approx_fast(out=rp, in_=pz)
                zp = keepf.tile([128, TP], FP, tag="zp", name=f"zp{c}")
                nc.gpsimd.tensor_tensor(out=zp, in0=ssum, in1=rp, op=AL.mult)
                wc_all[c], rs_all[c], zp_all[c] = wc, rs, zp

        emit_sqrt(NC - 1)
        # gate: all P2 exp ops depend on the last chunk's ds (zero bias)
        b_gate = const.tile([128, 1], FP)
        nc.vector.tensor_scalar(out=b_gate, in0=ds_all[NC - 1][:, 0:1],
                                scalar1=0.0, scalar2=0.0,
                                op0=AL.mult, op1=AL.add)

        # ------------- P2: softplus = ln(1+exp(.)) (ln_exp table) ----------
        nmm = (NC + NF * 3) * (TP // 512)
        mmi = 0
        for c in range(NC):
            full = c < NF
            t = tr2.tile([128, TP], FP, tag="t")
            nc.scalar.activation(t, ds_all[c], AF.Exp, scale=1.0 / SIGMA)
            l = keep.tile([128, TP], FP if full else BF, tag="l",
                          name=f"l{c}")
            nc.scalar.activation(l, t, AF.Ln, bias=b_one)
            if full:
                lb = tr2.tile([128, TP], BF, tag="lb")
                nc.scalar.activation(lb, l, AF.Copy)
                mv = lb
            else:
                mv = l
            for h in range(TP // 512):
                nc.tensor.matmul(acc[:, 512 * h:512 * (h + 1)],
                                 ln_st[:, c, :], mv[:, 512 * h:512 * (h + 1)],
                                 start=(mmi == 0), stop=(mmi == nmm - 1))
                mmi += 1
            if full:
                l_full[c] = l

        # ------------- P3: full-population rgb/dsum accumulation ----------
        for c in range(NF):
            u1 = fpool.tile([128, TP], FP, tag="u1")
            nc.vector.scalar_tensor_tensor(out=u1, in0=ds_all[c],
                                           scalar=1.0 / SIGMA, in1=l_full[c],
                                           op0=AL.mult, op1=AL.subtract)
            u2 = fpool.tile([128, TP], FP, tag="u2")
            nc.vector.scalar_tensor_tensor(out=u2, in0=zp_all[c],
                                           scalar=zsc, in1=u1,
                                           op0=AL.mult, op1=AL.add)
            pe = fpool.tile([128, TP], FP, tag="pe")
            nc.scalar.activation(pe, u2, AF.Exp, bias=b_pe)
            t0 = fpool.tile([128, TP], FP, tag="t0")
            nc.gpsimd.tensor_tensor(out=t0, in0=pe, in1=rs_all[c], op=AL.mult)
            for k in range(3):
                g = tr2.tile([128, TP], BF, tag="g", name=f"g{k}")
                nc.vector.tensor_tensor(out=g, in0=t0, in1=wc_all[c][:, k, :],
                                        op=AL.mult)
                for h in range(TP // 512):
                    nc.tensor.matmul(acc[:, 512 * h:512 * (h + 1)],
                                     g_st[:, c, k, :],
                                     g[:, 512 * h:512 * (h + 1)],
                                     start=(mmi == 0), stop=(mmi == nmm - 1))
                    mmi += 1

        # ---------------- P4: write out --------------------------------
        o = fpool.tile([ACC, TP], FP, tag="o")
        nc.scalar.activation(o, acc, AF.Copy)
        nc.sync.dma_start(out=d_out[:, :], in_=o)

    nc.compile()
    return nc


def kernel(face_vertices, face_textures):
    prep = _host_prep(face_vertices)
    mhat = float(max(prep['znUB'].max(), EPS))
    Tt, D = _tile_thresholds(prep, mhat)
    kf, ka, assign, NF, NA = _populate(prep, Tt, D, mhat)

    in_maps = []
    slots_all = []
    for c in range(NCORES):
        slots = _pack_slots(assign[c], kf, ka, NF, NA)
        slots_all.append(slots)
        in_maps.append(_build_inputs(prep, face_textures, slots, assign[c],
                                     NF, NA))

    nc = _build_program(NF, NA, mhat)
    global LAST_RESULT
    res = run_bass_kernel_spmd(nc, in_maps, core_ids=list(range(NCORES)),
                               trace=TRACE)
    LAST_RESULT = res

    out = np.zeros((1, 4, H, W), np.float32)
    wbg = np.exp((EPS - mhat) / GAMMA)
    for c in range(NCORES):
        o = np.asarray(res.results[c]["out"], np.float64)   # [80, TP]
        for tl, t in enumerate(assign[c]):
            ty, tx = t // NTX, t % NTX
            ys = slice(ty * TR, ty * TR + TR)
            xs = slice(tx * TC, tx * TC + TC)
            blk = o[5 * tl:5 * tl + 5]
            dsum = blk[3] + wbg
            rgb = blk[0:3] / np.maximum(dsum, 1e-37)[None]
            alpha = 1.0 - np.exp(-blk[4])
            out[0, 0:3, ys, xs] = rgb.reshape(3, TR, TC)
            out[0, 3, ys, xs] = alpha.reshape(TR, TC)
    return out
